# revision 46
# baseline (speedup 1.0000x reference)
"""VGCN encoder (2-layer GCN, shared normalized adjacency) on 8 Trainium2 cores.

Strategy: node-partitioned graph parallelism. Nodes are padded to
NPAD = 8*6272 and core c owns nodes [6272c, 6272(c+1)), split into 98 buckets
of 64. All edges (plus one self-edge per node, which realizes both GCN's +1
degree and the self-loop term) are routed to the core owning their dst node,
bucketed by dst bucket, and aggregated on-device with one-hot matmuls over
128-edge tiles (bf16 operands, fp32 psum):

    agg[bucket] += S.T @ us[src]   (S[e, j] = dst_local[e] == j, built on DVE)

Messages are fetched with SWDGE dma_gather (4 queues round-robin) from a
bf16 DRAM table whose 256-byte rows ([HID bf16 payload | HID pad], the
minimum SWDGE element) are stored in a (core, partition, bucket)-major
permutation so bulk table writes are contiguous DMAs; the host permutes
gather indices to match. dis = 1/sqrt(deg) is precomputed on the host (pure
graph structure, like the edge bucketing itself).

Layer 1 input (x @ W1, x shipped in bf16) is computed REPLICATED on every
core for the whole graph - it is ~50 MFLOP, far cheaper than an AllGather
round - so the only collective is the AllGather of the layer-1 activations
(packed bf16, Shared-scratchpad output, expanded to padded rows locally).
Layer-2 aggregation runs feature-major (lhsT=msg) so both output projections
become 13 wide matmuls against [Wmu | Wlv] with one transpose per 128 nodes.

All host-prepared operands travel in ONE packed int16 input tensor (f32/bf16
sections bitcast on device) and both outputs leave in ONE tensor; together
with pipelined dispatch this keeps the per-execution dispatch overhead of the
axon tunnel (~0.8 ms flat per round trip) mostly off the measured
steady-state time.
"""

import sys

sys.path.insert(0, "/opt/trn_rl_repo")

import numpy as np

from concourse import bacc, mybir, tile
from concourse.bass_utils import run_bass_kernel_spmd
from concourse.masks import make_identity

F32 = mybir.dt.float32
BF16 = mybir.dt.bfloat16
I16 = mybir.dt.int16
I32 = mybir.dt.int32


class Cfg:
    def __init__(self, n=50000, e=800000, in_dim=128, hid=64, ncores=8,
                 shard_tiles=49, bw=64, half=32768, chunk_tiles=32, sbatch=32):
        self.N, self.E, self.IN, self.HID = n, e, in_dim, hid
        self.NCORES = ncores
        self.P = 128
        self.SHARD = shard_tiles * 128    # nodes per core
        self.NPAD = ncores * self.SHARD
        self.BW = bw                      # bucket width (psum partition dim)
        self.NBK = self.SHARD // bw       # buckets per core
        self.GBK = ncores * self.NBK      # global buckets
        self.HALF = half                  # gather-table split so int16 idx fit
        self.CH = chunk_tiles             # tiles (128 rows) per dma_gather
        self.SB = sbatch                  # tiles per batched one-hot build
        assert self.NPAD >= n and half <= 32768 and self.SHARD % bw == 0
        assert self.NBK <= 128 and self.NBK % 2 == 0


DEFAULT = Cfg()


def pack_offsets(cfg, nTA, nTB, nT2, ncol2):
    """Column offsets (int16 units) of each section in the packed input."""
    off, out = 0, {}
    for name, width in (("xT", cfg.NPAD), ("w1", 2 * cfg.HID),
                        ("wml", 4 * cfg.HID), ("dis", 2 * cfg.NBK),
                        ("disP", cfg.NBK), ("dlA", nTA),
                        ("dlB", nTB), ("dl2", ncol2),
                        ("idxA", 8 * nTA), ("idxB", 8 * nTB),
                        ("idx2", 8 * nT2)):
        out[name] = off
        off += width
    out["W"] = off
    return out


def build_layout(edge_index, cfg=DEFAULT):
    """Static per-core edge streams plus the (identical-across-cores) tile
    structure. Table row of node n: c*SHARD + (r%BW)*NBK + r//BW, r=n%SHARD."""
    src = np.asarray(edge_index[0], np.int64)
    dst = np.asarray(edge_index[1], np.int64)
    NBK, BW = cfg.NBK, cfg.BW

    deg = np.bincount(dst, minlength=cfg.NPAD).astype(np.float64) + 1.0
    dis = (1.0 / np.sqrt(deg)).astype(np.float32)   # padding nodes: dis=1

    per_core = []
    cnts = np.zeros((cfg.NCORES, NBK * 2), np.int64)
    for c in range(cfg.NCORES):
        m = (dst >= c * cfg.SHARD) & (dst < (c + 1) * cfg.SHARD)
        s = src[m]
        d = dst[m]
        selfn = np.arange(c * cfg.SHARD, (c + 1) * cfg.SHARD, dtype=np.int64)
        s = np.concatenate([s, selfn])
        d = np.concatenate([d, selfn])
        cc, rr = s // cfg.SHARD, s % cfg.SHARD
        row = cc * cfg.SHARD + (rr % BW) * NBK + rr // BW
        dr = d - c * cfg.SHARD
        b = dr // BW
        dl = dr % BW
        h = (row >= cfg.HALF).astype(np.int64)
        key = b * 2 + h
        order = np.argsort(key, kind="stable")
        row, dl, key = row[order], dl[order], key[order]
        per_core.append((row, dl, key))
        cnts[c] = np.bincount(key, minlength=NBK * 2)

    ntile = np.ceil(cnts.max(axis=0) / 128.0).astype(np.int64)
    ntA, ntB = ntile[0::2], ntile[1::2]
    nTA, nTB = int(ntA.sum()), int(ntB.sum())

    tbA = np.repeat(np.arange(NBK), ntA)
    tbB = np.repeat(np.arange(NBK), ntB)
    offA = np.concatenate([[0], np.cumsum(ntA)]) * 128
    offB = np.concatenate([[0], np.cumsum(ntB)]) * 128

    cores = []
    for c in range(cfg.NCORES):
        row, dl, key = per_core[c]
        bounds = np.searchsorted(key, np.arange(NBK * 2 + 1))
        idxA = np.zeros(nTA * 128, np.int64)
        dlA = np.full(nTA * 128, BW, np.int64)
        idxB = np.zeros(nTB * 128, np.int64)
        dlB = np.full(nTB * 128, BW, np.int64)
        for b in range(NBK):
            lo, hi = bounds[2 * b], bounds[2 * b + 1]
            o = offA[b]
            idxA[o:o + hi - lo] = row[lo:hi]
            dlA[o:o + hi - lo] = dl[lo:hi]
            lo, hi = bounds[2 * b + 1], bounds[2 * b + 2]
            o = offB[b]
            idxB[o:o + hi - lo] = row[lo:hi] - cfg.HALF
            dlB[o:o + hi - lo] = dl[lo:hi]

        def wrap(stream):
            a = stream.reshape(-1, 16).T.astype(np.int16)
            return np.tile(a, (8, 1))   # replicated across the 8 q7 cores

        # dis for own nodes in (partition=dl, bucket) layout
        dis_own = np.ascontiguousarray(
            dis[c * cfg.SHARD:(c + 1) * cfg.SHARD].reshape(NBK, BW).T)

        disP = np.ascontiguousarray(
            dis_own.reshape(BW, NBK // 2, 2).transpose(2, 0, 1)
            .reshape(2 * BW, NBK // 2))

        cores.append(dict(
            idxA=wrap(idxA), idxB=wrap(idxB),
            dlA=np.ascontiguousarray(dlA.reshape(-1, 128).T.astype(np.float32)),
            dlB=np.ascontiguousarray(dlB.reshape(-1, 128).T.astype(np.float32)),
            dis=dis_own, disP=disP,
        ))

    # ---- layer 2: src-local streams grouped by (bucket-octet, dst core) ---
    # Core c owns hs2 for its shard only (no AllGather); it aggregates
    # feature-major PARTIALS for every core's buckets from the edges whose
    # src lives on c, and a ReduceScatter sums the partials at the end.
    OCT = [(q, min(q + 8, NBK)) for q in range(0, NBK, 8)]
    GD = [(gi, dc) for gi in range(len(OCT)) for dc in range(cfg.NCORES)]
    per2 = []
    cnt2 = np.zeros((cfg.NCORES, len(GD)), np.int64)
    for c in range(cfg.NCORES):
        # NO self-edges here: the layer-2 self term is added post-RS as
        # aggT += hs2T (self-edges would inflate the diagonal groups' max
        # tile counts by ~30%).
        m = (src >= c * cfg.SHARD) & (src < (c + 1) * cfg.SHARD)
        s = src[m] - c * cfg.SHARD
        d = dst[m]
        row2 = (s % BW) * NBK + s // BW          # local hs2 table row
        dc_ = d // cfg.SHARD
        dr = d % cfg.SHARD
        db = dr // BW                             # dst bucket on core dc_
        dj = dr % BW                              # dst lane within bucket
        gi = db // 8
        key = (gi * cfg.NCORES + dc_) * NBK + db
        order = np.argsort(key, kind="stable")
        per2.append((row2[order], dj[order], db[order],
                     (gi * cfg.NCORES + dc_)[order]))
        cnt2[c] = np.bincount((gi * cfg.NCORES + dc_)[order],
                              minlength=len(GD))

    nt2 = np.maximum(np.ceil(cnt2.max(axis=0) / 128.0).astype(np.int64), 1)
    off2 = np.concatenate([[0], np.cumsum(nt2)])
    nT2 = int(nt2.sum())

    # per (group, tile): union over cores of sub-buckets present -> S columns
    # (one matmul per column; per-core dl marks its edges, 64 elsewhere)
    ent2 = []    # per group: list of (tile, bsub, col)
    col = 0
    tile_bsubs = []
    for g, (gi, dc) in enumerate(GD):
        q0, q1 = OCT[gi]
        per_tile = [set() for _ in range(int(nt2[g]))]
        for c in range(cfg.NCORES):
            row2, dj, db, key = per2[c]
            lo, hi = np.searchsorted(key, [g, g + 1])
            for t in range(int(nt2[g])):
                sl = db[lo + t * 128: lo + min((t + 1) * 128, hi - lo)]
                per_tile[t].update(int(b) - q0 for b in np.unique(sl))
        # every psum slot [0, q1-q0) must be written at least once; give
        # slots with no edges anywhere a dummy all-dead column (S == 0)
        present = set().union(*per_tile) if per_tile else set()
        for bsub in range(q1 - q0):
            if bsub not in present:
                per_tile[0].add(bsub)
        glist = []
        for t, bs in enumerate(per_tile):
            for bsub in sorted(bs):
                glist.append((t, bsub, col))
                col += 1
        ent2.append(glist)
        tile_bsubs.append(per_tile)
    ncol2 = col

    for c in range(cfg.NCORES):
        row2, dj, db, key = per2[c]
        idx2 = np.zeros(nT2 * 128, np.int64)
        dl2 = np.full((128, ncol2), BW, np.int64)
        for g, (gi, dc) in enumerate(GD):
            q0, q1 = OCT[gi]
            lo, hi = np.searchsorted(key, [g, g + 1])
            n = hi - lo
            o = off2[g] * 128
            idx2[o:o + n] = row2[lo:hi]
            for t, bsub, k in ent2[g]:
                t0 = lo + t * 128
                t1 = lo + min((t + 1) * 128, n)
                if t1 <= t0:
                    continue
                pos = np.arange(t0, t1)
                sel = db[pos] - q0 == bsub
                dl2[(pos - t0)[sel], k] = dj[pos[sel]]
        cores[c]["idx2"] = np.tile(
            idx2.reshape(-1, 16).T.astype(np.int16), (8, 1))
        cores[c]["dl2"] = dl2.astype(np.float32)

    return dict(ntA=tuple(int(x) for x in ntA), ntB=tuple(int(x) for x in ntB),
                tbA=tbA, tbB=tbB, nTA=nTA, nTB=nTB, cores=cores,
                dis_full=dis.astype(np.float32),
                OCT=OCT, GD=GD, nt2=tuple(int(x) for x in nt2),
                off2=off2, nT2=nT2, ent2=ent2, ncol2=ncol2)


def build_program(layout, cfg=DEFAULT, has_bias=False, reps=1,
                  skip_cc=False, phases=3, gather_mode="gather", proj=True):
    """Emit the SPMD bass program (identical on all cores)."""
    nc = bacc.Bacc("TRN2", target_bir_lowering=False, debug=False,
                   num_devices=cfg.NCORES, num_swdge_queues=4)
    P, BW, NBK, HID = cfg.P, cfg.BW, cfg.NBK, cfg.HID
    nTA, nTB = layout["nTA"], layout["nTB"]
    tb = {0: layout["tbA"], 1: layout["tbB"]}
    nT = {0: nTA, 1: nTB}
    HALVES = [H for H in (0, 1) if nT[H] > 0]
    use_cc = cfg.NCORES > 1 and not skip_cc
    OCT, GD, ent2 = layout["OCT"], layout["GD"], layout["ent2"]
    nt2, off2, nT2, ncol2 = (layout["nt2"], layout["off2"],
                             layout["nT2"], layout["ncol2"])

    # ---------------- I/O (single packed input, single output) ----------
    OFF = pack_offsets(cfg, nTA, nTB, nT2, ncol2)
    pk_in = nc.dram_tensor("pk", [P, OFF["W"]], I16, kind="ExternalInput")

    def fsec(name, n, rows=P):
        o = OFF[name]
        return pk_in.ap()[:rows, o:o + 2 * n].bitcast(F32)

    def bsec(name, n, rows=P):
        o = OFF[name]
        return pk_in.ap()[:rows, o:o + n].bitcast(BF16)

    def isec(name, n):
        o = OFF[name]
        return pk_in.ap()[:, o:o + n]

    xT_b = pk_in.ap()[:, OFF["xT"]:OFF["xT"] + cfg.NPAD].bitcast(BF16)
    if has_bias:
        b1_in = nc.dram_tensor("b1", [1, HID], F32, kind="ExternalInput")
        bmu_in = nc.dram_tensor("bmu", [1, HID], F32, kind="ExternalInput")
        blv_in = nc.dram_tensor("blv", [1, HID], F32, kind="ExternalInput")
    z_out = nc.dram_tensor("z", [P, NBK // 2, 2 * HID], BF16,
                           kind="ExternalOutput")

    with tile.TileContext(nc) as tc:
        import contextlib
        stack = contextlib.ExitStack()
        with stack:
            dram = stack.enter_context(tc.tile_pool(name="dram", bufs=1, space="DRAM"))
            cpool = stack.enter_context(tc.tile_pool(name="const", bufs=1))

            us_tab = dram.tile([cfg.NPAD, 2 * HID], BF16)
            # layer-1 activations stay LOCAL: hs2loc is this core's gather
            # table for layer 2; partial aggregates for every core's nodes
            # are summed by chunked ReduceScatters at the end.
            hs2loc = dram.tile([cfg.SHARD, 2 * HID], BF16)
            CHUNKS = [(b, min(b + 26, NBK)) for b in range(0, NBK, 26)]
            # ReduceScatter chunks in octet-group units -> node-column ranges
            RSC = [(0, 4), (4, 7), (7, 10), (10, 13)]
            RSC = [(g0, g1, OCT[g0][0] * BW, OCT[g1 - 1][1] * BW)
                   for g0, g1 in RSC]
            partial = [dram.tile([cfg.NCORES * HID, c1 - c0], BF16,
                                 tag=f"par{c0}", name=f"partial{c0}")
                       for _, _, c0, c1 in RSC]
            aggT_d = [dram.tile([HID, c1 - c0], BF16,
                                tag=f"agg{c0}", name=f"aggT{c0}")
                      for _, _, c0, c1 in RSC]

            w1_f = cpool.tile([cfg.IN, HID], F32)
            nc.sync.dma_start(out=w1_f[:], in_=fsec("w1", HID))
            w1_sb = cpool.tile([cfg.IN, HID], BF16)
            nc.vector.tensor_copy(out=w1_sb[:], in_=w1_f[:])
            wml_f = cpool.tile([HID, 2 * HID], F32)
            nc.sync.dma_start(out=wml_f[:], in_=fsec("wml", 2 * HID, HID))
            wml_sb = cpool.tile([HID, 2 * HID], BF16)
            nc.vector.tensor_copy(out=wml_sb[:], in_=wml_f[:])
            dis_own = cpool.tile([BW, NBK], F32)
            nc.sync.dma_start(out=dis_own[:], in_=fsec("dis", NBK, BW))
            disP = cpool.tile([P, NBK // 2], F32)
            nc.sync.dma_start(out=disP[:], in_=fsec("disP", NBK // 2))
            dis2 = cpool.tile([BW, NBK], F32)
            nc.vector.tensor_tensor(out=dis2[:], in0=dis_own[:],
                                    in1=dis_own[:], op=mybir.AluOpType.mult)

            iota_i = cpool.tile([P, BW], I32)
            nc.gpsimd.iota(iota_i[:], pattern=[[1, BW]], base=0,
                           channel_multiplier=0)
            iota_b = cpool.tile([P, BW], BF16)
            nc.vector.tensor_copy(out=iota_b[:], in_=iota_i[:])

            ident = cpool.tile([P, P], F32)
            make_identity(nc, ident[:])
            ident_bf = cpool.tile([P, P], BF16)
            nc.vector.tensor_copy(out=ident_bf[:], in_=ident[:])

            idx_sb, dl_sb = {}, {}
            for H in HALVES:
                idx_sb[H] = cpool.tile([P, nT[H] * 8], I16, tag=f"idx{H}",
                                       name=f"idx{H}")
                nc.sync.dma_start(out=idx_sb[H][:],
                                  in_=isec("idxA" if H == 0 else "idxB",
                                           nT[H] * 8))
                dl_sb[H] = cpool.tile([P, nT[H]], BF16, tag=f"dl{H}",
                                      name=f"dls{H}")
                nc.sync.dma_start(out=dl_sb[H][:],
                                  in_=bsec("dlA" if H == 0 else "dlB",
                                           nT[H]))
            idx2_sb = cpool.tile([P, nT2 * 8], I16, tag="idx2", name="idx2")
            nc.sync.dma_start(out=idx2_sb[:], in_=isec("idx2", nT2 * 8))
            dl2_sb = cpool.tile([P, ncol2], BF16, tag="dl2", name="dl2")
            nc.sync.dma_start(out=dl2_sb[:], in_=bsec("dl2", ncol2))

            if has_bias:
                brow = cpool.tile([1, 3 * HID], F32)
                nc.sync.dma_start(out=brow[:, 0:HID], in_=b1_in.ap()[:])
                nc.sync.dma_start(out=brow[:, HID:2 * HID], in_=bmu_in.ap()[:])
                nc.sync.dma_start(out=brow[:, 2 * HID:], in_=blv_in.ap()[:])
                bias_bc = cpool.tile([P, 3 * HID], F32)
                nc.gpsimd.partition_broadcast(bias_bc[:], brow[:])

            def build_S(spool, H, tag):
                tiles = []
                for t0 in range(0, nT[H], cfg.SB):
                    tn = min(cfg.SB, nT[H] - t0)
                    st = spool.tile([P, cfg.SB, BW], BF16, tag=tag,
                                    name=f"S{tag}")
                    nc.vector.tensor_tensor(
                        out=st[:, :tn, :],
                        in0=dl_sb[H][:, t0:t0 + tn].to_broadcast([P, tn, BW]),
                        in1=iota_b[:, None, :].to_broadcast([P, tn, BW]),
                        op=mybir.AluOpType.is_equal,
                    )
                    tiles.append(st)

                def one(t):
                    return tiles[t // cfg.SB][:, t % cfg.SB, :]

                return one

            qctr = [0]

            def gather_all(mpool, table, tagp):
                """Gather both halves' chunks, interleaved so the earliest
                tiles of BOTH halves land first (bucket 0 needs both)."""
                tiles = {H: [] for H in HALVES}
                starts = {H: list(range(0, nT[H], cfg.CH)) for H in HALVES}
                order = []
                for i in range(max(len(starts[H]) for H in HALVES)):
                    for H in HALVES:
                        if i < len(starts[H]):
                            order.append((H, starts[H][i]))
                for H, t0 in order:
                    tn = min(cfg.CH, nT[H] - t0)
                    mt = mpool.tile([P, cfg.CH, 2 * HID], BF16,
                                    tag=f"{tagp}{H}", name=f"M{tagp}{H}")
                    if gather_mode == "copy":
                        nc.sync.dma_start(
                            out=mt[:, :tn, :],
                            in_=table[:tn * 128, :]
                            .rearrange("(t p) f -> p t f", p=128))
                    else:
                        nc.gpsimd.dma_gather(
                            out_ap=mt[:, :tn, :],
                            in_ap=(table[:min(cfg.HALF, cfg.NPAD), :]
                                   if H == 0 else table[cfg.HALF:, :]),
                            idxs_ap=idx_sb[H][:, t0 * 8:(t0 + tn) * 8],
                            num_idxs=tn * 128, num_idxs_reg=tn * 128,
                            elem_size=2 * HID,
                            single_packet=(tn * 128 <= 512),
                            queue_num=qctr[0] % 4,
                        )
                        qctr[0] += 1
                    tiles[H].append(mt)

                def make(H):
                    tl = tiles[H]
                    return lambda t: tl[t // cfg.CH][:, t % cfg.CH, 0:HID]

                return {H: make(H) for H in HALVES}

            entries = [[] for _ in range(NBK)]
            for H in HALVES:
                for t, b in enumerate(tb[H]):
                    entries[int(b)].append((H, t))

            for _rep in range(reps):
                # ========= PHASE A: u = x @ W1, scaled by dis -> us table ====
                it_stack = contextlib.ExitStack()
                with it_stack:
                    xa = it_stack.enter_context(tc.tile_pool(name="xa", bufs=2))
                    pu = it_stack.enter_context(
                        tc.tile_pool(name="pu", bufs=2, space="PSUM"))
                    usb = it_stack.enter_context(tc.tile_pool(name="usb", bufs=2))
                    spool = it_stack.enter_context(tc.tile_pool(name="spool", bufs=2))
                    mpool = it_stack.enter_context(tc.tile_pool(name="mpool", bufs=4))
                    pagg = it_stack.enter_context(
                        tc.tile_pool(name="pagg", bufs=3, space="PSUM"))
                    hb = it_stack.enter_context(tc.tile_pool(name="hb", bufs=2))
                    pproj = it_stack.enter_context(
                        tc.tile_pool(name="pproj", bufs=2, space="PSUM"))
                    pz = it_stack.enter_context(
                        tc.tile_pool(name="pz", bufs=1, space="PSUM"))

                    XC = 8   # buckets per psum bank / ACT copy batch
                    XL = 49  # buckets per xT DMA (2 loads per shard)
                    if phases < 1:
                        zfill0 = usb.tile([BW, NBK, HID], BF16, tag="usbb",
                                          name="zfill0")
                        nc.vector.memset(zfill0[:], 0)
                    c2_order = [5, 6, 7, 0, 1, 2, 3, 4][:cfg.NCORES]
                    for c2 in (c2_order if phases >= 1 else []):
                        us_blk = usb.tile([BW, NBK, HID], BF16, tag="usbb",
                                          name="us_blk")
                        xtbs = {}
                        for L0 in range(0, NBK, XL):
                            ln = min(XL, NBK - L0)
                            xtb = xa.tile([P, XL, BW], BF16, tag="xtb",
                                          name="xtb")
                            nc.sync.dma_start(
                                out=xtb[:, :ln, :],
                                in_=xT_b[:, c2 * cfg.SHARD + L0 * BW:
                                         c2 * cfg.SHARD + (L0 + ln) * BW]
                                .rearrange("p (t q) -> p t q", t=ln))
                            xtbs[L0] = xtb
                        for B0 in range(0, NBK, XC):
                            bn = min(XC, NBK - B0)
                            ups = pu.tile([BW, XC, HID], F32, space="PSUM",
                                          tag="u", name="ups")
                            for j in range(bn):
                                L0 = ((B0 + j) // XL) * XL
                                nc.tensor.matmul(out=ups[:, j, :],
                                                 lhsT=xtbs[L0][:, B0 + j - L0, :],
                                                 rhs=w1_sb[:],
                                                 start=True, stop=True)
                            # dis[src] is folded into x on the host, so this
                            # is a pure psum->sbuf bf16 cast (ACT engine).
                            nc.scalar.copy(out=us_blk[:, B0:B0 + bn, :],
                                           in_=ups[:, :bn, :])
                        nc.sync.dma_start(
                            out=us_tab[c2 * cfg.SHARD:(c2 + 1) * cfg.SHARD,
                                       0:HID]
                            .rearrange("(j b) f -> j b f", j=BW),
                            in_=us_blk[:])

                    # ================= PHASE B: layer-1 aggregation =============
                    if phases >= 2:
                        msg = gather_all(mpool, us_tab[:], "m")
                        S1 = {H: build_S(spool, H, f"s{H}") for H in HALVES}
                        tabv = hs2loc[:, 0:HID].rearrange(
                            "(j b) f -> j b f", j=BW)
                        hs2_chunks = []
                        for ci, (B0, B1) in enumerate(CHUNKS):
                            hs2_sb = usb.tile([BW, B1 - B0, HID], BF16,
                                              tag=f"hsc{ci}", bufs=1,
                                              name=f"hs2_sb{ci}")
                            hs2_chunks.append(hs2_sb)
                            for b0 in range(B0, B1, 2):
                                ps = pagg.tile([BW, 2, HID], F32, space="PSUM",
                                               tag="agg", name="ps1")
                                for k in (0, 1):
                                    ent = entries[b0 + k]
                                    for i, (H, t) in enumerate(ent):
                                        nc.tensor.matmul(
                                            out=ps[:, k, :], lhsT=S1[H](t),
                                            rhs=msg[H](t), start=(i == 0),
                                            stop=(i == len(ent) - 1))
                                bl = b0 - B0
                                if has_bias:
                                    t1 = hb.tile([BW, 2, HID], F32, tag="h",
                                                 name="t1")
                                    # h1 = relu(ps*dis + b); hs2 = h1*dis
                                    dpair = dis_own[:, b0:b0 + 2, None] \
                                        .to_broadcast([BW, 2, HID])
                                    nc.vector.tensor_tensor(
                                        out=t1[:], in0=ps[:], in1=dpair,
                                        op=mybir.AluOpType.mult)
                                    nc.vector.tensor_tensor(
                                        out=t1[:], in0=t1[:],
                                        in1=bias_bc[:BW, None, 0:HID]
                                        .to_broadcast([BW, 2, HID]),
                                        op=mybir.AluOpType.add)
                                    nc.vector.tensor_relu(out=t1[:], in_=t1[:])
                                    nc.vector.tensor_tensor(
                                        out=hs2_sb[:, bl:bl + 2, :], in0=t1[:],
                                        in1=dpair, op=mybir.AluOpType.mult)
                                else:
                                    # dis>0: relu(ps)*dis^2 == relu(ps*dis^2),
                                    # one ACT op per bucket (keeps DVE free
                                    # for the S-matrix builds).
                                    for k in (0, 1):
                                        nc.scalar.activation(
                                            out=hs2_sb[:, bl + k, :],
                                            in_=ps[:, k, :],
                                            func=mybir.ActivationFunctionType.Relu,
                                            scale=dis2[:, b0 + k, None])
                            nc.sync.dma_start(out=tabv[:, B0:B1, :],
                                              in_=hs2_sb[:])

                    # ============== PHASE C: layer-2 + projections ==============
                    if phases >= 3:
                        # hs2T = transpose of this core's hs2 (node-major ->
                        # feature-major) for the post-RS self-loop term.
                        hs2T = usb.tile([HID, NBK * BW], BF16, tag="h2t",
                                        bufs=1, name="hs2T")
                        for ci, (B0, B1) in enumerate(CHUNKS):
                            for b in range(B0, B1):
                                tp = pz.tile([P, P], BF16, space="PSUM",
                                             tag="z", name="tp")
                                nc.tensor.transpose(
                                    out=tp[:HID, :BW],
                                    in_=hs2_chunks[ci][:, b - B0, :],
                                    identity=ident_bf[:BW, :BW])
                                nc.scalar.copy(
                                    out=hs2T[:, b * BW:(b + 1) * BW],
                                    in_=tp[:HID, :BW])
                        # gather layer-2 messages from the LOCAL hs2 table
                        # (starts as soon as phase B finishes; no collective
                        # in the way).
                        m2tiles = []
                        for ti, t0 in enumerate(range(0, nT2, cfg.CH)):
                            tn = min(cfg.CH, nT2 - t0)
                            mt = mpool.tile([P, cfg.CH, 2 * HID], BF16,
                                            tag=f"m{ti % 2}", name="M2")
                            nc.gpsimd.dma_gather(
                                out_ap=mt[:, :tn, :],
                                in_ap=hs2loc[:],
                                idxs_ap=idx2_sb[:, t0 * 8:(t0 + tn) * 8],
                                num_idxs=tn * 128, num_idxs_reg=tn * 128,
                                elem_size=2 * HID,
                                single_packet=(tn * 128 <= 512),
                                queue_num=qctr[0] % 4,
                            )
                            qctr[0] += 1
                            m2tiles.append(mt)

                        def msg2(t):
                            return m2tiles[t // cfg.CH][:, t % cfg.CH, 0:HID]

                        s2tiles = []
                        for k0 in range(0, ncol2, cfg.SB):
                            kn = min(cfg.SB, ncol2 - k0)
                            st = spool.tile([P, cfg.SB, BW], BF16, tag="s0",
                                            name="S2t")
                            nc.vector.tensor_tensor(
                                out=st[:, :kn, :],
                                in0=dl2_sb[:, k0:k0 + kn]
                                .to_broadcast([P, kn, BW]),
                                in1=iota_b[:, None, :].to_broadcast([P, kn, BW]),
                                op=mybir.AluOpType.is_equal,
                            )
                            s2tiles.append(st)

                        def S2(k):
                            return s2tiles[k // cfg.SB][:, k % cfg.SB, :]

                        # feature-major partials per (octet, dst core);
                        # ReduceScatter chunk fires once its octets are done.
                        rs_k = 0
                        for gi, (q0, q1) in enumerate(OCT):
                            gw = q1 - q0
                            for dcc in range(cfg.NCORES):
                                g = gi * cfg.NCORES + dcc
                                tbase = int(off2[g])
                                ps = pagg.tile([HID, 8, BW], F32, space="PSUM",
                                               tag="agg", name="ps2")
                                slots = {}
                                for (t, bsub, k) in ent2[g]:
                                    slots.setdefault(bsub, []).append((t, k))
                                for bsub in sorted(slots):
                                    lst = slots[bsub]
                                    for i, (t, k) in enumerate(lst):
                                        nc.tensor.matmul(
                                            out=ps[:, bsub, :],
                                            lhsT=msg2(tbase + t), rhs=S2(k),
                                            start=(i == 0),
                                            stop=(i == len(lst) - 1))
                                stg = hb.tile([HID, 8, BW], BF16, tag="stg",
                                              bufs=3, name="stg")
                                nc.scalar.copy(out=stg[:, :gw, :],
                                               in_=ps[:, :gw, :])
                                g0, g1, c0, c1 = RSC[rs_k]
                                eng = nc.sync if dcc % 2 == 0 else nc.scalar
                                eng.dma_start(
                                    out=partial[rs_k][dcc * HID:
                                                      (dcc + 1) * HID,
                                                      q0 * BW - c0:
                                                      q1 * BW - c0]
                                    .rearrange("f (b j) -> f b j", b=gw),
                                    in_=stg[:, :gw, :])
                            g0, g1, c0, c1 = RSC[rs_k]
                            if gi == g1 - 1:
                                if use_cc:
                                    nc.gpsimd.collective_compute(
                                        "ReduceScatter",
                                        mybir.AluOpType.add,
                                        replica_groups=[
                                            list(range(cfg.NCORES))],
                                        ins=[partial[rs_k].opt()],
                                        outs=[aggT_d[rs_k].opt()],
                                    )
                                else:
                                    nc.sync.dma_start(
                                        out=aggT_d[rs_k][:],
                                        in_=partial[rs_k][0:HID, :])
                                rs_k += 1

                        # per-RS-chunk projection + transpose + dis scale
                        for k, (g0, g1, c0, c1) in enumerate(RSC):
                            cw = c1 - c0
                            aggk = usb.tile([HID, 2048], BF16, tag="agg2",
                                            name="aggk")
                            nc.sync.dma_start(out=aggk[:, :cw],
                                              in_=aggT_d[k][:])
                            # self-loop term: agg[n] += hs2[n] for own nodes
                            nc.vector.tensor_tensor(
                                out=aggk[:, :cw], in0=aggk[:, :cw],
                                in1=hs2T[:, c0:c1],
                                op=mybir.AluOpType.add)
                            zT_sb = usb.tile([2 * HID, 2048], BF16,
                                             tag="zt", name="zT_sb")
                            for n0 in range(0, cw, 512):
                                cn = min(512, cw - n0)
                                zT_ps = pproj.tile([2 * HID, 512], F32,
                                                   space="PSUM", tag="zT",
                                                   name="zT_ps")
                                nc.tensor.matmul(out=zT_ps[:, :cn],
                                                 lhsT=wml_sb[:],
                                                 rhs=aggk[:, n0:n0 + cn],
                                                 start=True, stop=True)
                                nc.scalar.copy(out=zT_sb[:, n0:n0 + cn],
                                               in_=zT_ps[:, :cn])
                            zcat_sb = usb.tile([P, 16, 2 * HID], BF16,
                                               tag="zc", name="zcat_sb")
                            t0, t1 = c0 // P, c1 // P
                            for t in range(t0, t1):
                                z_ps = pz.tile([P, P], BF16, space="PSUM",
                                               tag="z", name="z_ps")
                                nc.tensor.transpose(
                                    out=z_ps[:],
                                    in_=zT_sb[:, (t - t0) * P:
                                              (t - t0 + 1) * P],
                                    identity=ident_bf[:])
                                nc.scalar.mul(out=zcat_sb[:, t - t0, :],
                                              in_=z_ps[:],
                                              mul=disP[:, t, None])
                            if has_bias:
                                nc.vector.tensor_tensor(
                                    out=zcat_sb[:, :t1 - t0, :],
                                    in0=zcat_sb[:, :t1 - t0, :],
                                    in1=bias_bc[:, None, HID:3 * HID]
                                    .to_broadcast([P, t1 - t0, 2 * HID]),
                                    op=mybir.AluOpType.add)
                            nc.sync.dma_start(
                                out=z_out.ap()[:, t0:t1, :],
                                in_=zcat_sb[:, :t1 - t0, :])
                    if phases < 3:
                        zfill = usb.tile([P, NBK // 2, 2 * HID], BF16,
                                         tag="usb", name="zfill")
                        nc.vector.memset(zfill[:], 0)
                        nc.sync.dma_start(out=z_out.ap()[:], in_=zfill[:])

    nc.compile()
    return nc


_CACHE = {}


def _get_program(edge_index, cfg, has_bias):
    layout = build_layout(edge_index, cfg)
    key = (layout["ntA"], layout["ntB"], layout["nT2"], layout["ncol2"],
           has_bias)
    if key not in _CACHE:
        _CACHE[key] = build_program(layout, cfg, has_bias)
    return _CACHE[key], layout


def make_in_maps(x, edge_index, W1, b1, Wmu, bmu, Wlv, blv, layout,
                 cfg=DEFAULT, has_bias=False):
    x = np.asarray(x, np.float32)
    xpad = np.zeros((cfg.NPAD, cfg.IN), np.float32)
    xpad[:x.shape[0]] = x
    # fold dis[src] into x so phase A's matmul directly yields us = dis*(x@W1)
    xpad *= layout["dis_full"][:, None]
    xT = np.ascontiguousarray(xpad.T)
    wml = np.concatenate([np.asarray(Wmu, np.float32),
                          np.asarray(Wlv, np.float32)], axis=1)
    w1 = np.asarray(W1, np.float32)
    nTA, nTB = layout["nTA"], layout["nTB"]
    nT2, ncol2 = layout["nT2"], layout["ncol2"]
    OFF = pack_offsets(cfg, nTA, nTB, nT2, ncol2)

    def put_f32(pk, name, arr):
        arr = np.asarray(arr, np.float32)
        o = OFF[name]
        pk[:arr.shape[0], o:o + 2 * arr.shape[1]] = arr.view(np.int16)

    def put_bf16(pk, name, arr):
        import ml_dtypes
        arr = np.asarray(arr, np.float32).astype(ml_dtypes.bfloat16)
        o = OFF[name]
        pk[:arr.shape[0], o:o + arr.shape[1]] = arr.view(np.int16)

    maps = []
    for c in range(cfg.NCORES):
        pk = np.zeros((cfg.P, OFF["W"]), np.int16)
        cd = layout["cores"][c]
        put_bf16(pk, "xT", xT)
        put_f32(pk, "w1", w1)
        put_f32(pk, "wml", wml)
        put_f32(pk, "dis", cd["dis"])
        put_f32(pk, "disP", cd["disP"])
        put_bf16(pk, "dlA", cd["dlA"])
        put_bf16(pk, "dlB", cd["dlB"])
        put_bf16(pk, "dl2", cd["dl2"])
        pk[:, OFF["idxA"]:OFF["idxA"] + 8 * nTA] = cd["idxA"]
        pk[:, OFF["idxB"]:OFF["idxB"] + 8 * nTB] = cd["idxB"]
        pk[:, OFF["idx2"]:OFF["idx2"] + 8 * nT2] = cd["idx2"]
        m = dict(pk=pk)
        if has_bias:
            m.update(b1=np.asarray(b1, np.float32).reshape(1, -1),
                     bmu=np.asarray(bmu, np.float32).reshape(1, -1),
                     blv=np.asarray(blv, np.float32).reshape(1, -1))
        maps.append(m)
    return maps


def unshard(results, cfg=DEFAULT):
    H = cfg.HID
    zmu_blocks, zlv_blocks = [], []
    for c in range(cfg.NCORES):
        z = np.asarray(results[c]["z"]).astype(np.float32)
        z4 = z.reshape(2, cfg.BW, cfg.NBK // 2, 2 * H)
        zjb = np.transpose(z4, (2, 0, 1, 3)).reshape(cfg.NBK, cfg.BW, 2 * H)
        # zjb[b, j, :]: node c*SHARD + b*BW + j
        zmu_blocks.append(zjb[:, :, 0:H].reshape(cfg.SHARD, H))
        zlv_blocks.append(zjb[:, :, H:2 * H].reshape(cfg.SHARD, H))
    return (np.concatenate(zmu_blocks, axis=0)[:cfg.N],
            np.concatenate(zlv_blocks, axis=0)[:cfg.N])


def kernel(x, edge_index, W1, b1, Wmu, bmu, Wlv, blv):
    cfg = DEFAULT
    has_bias = any(np.any(np.asarray(b)) for b in (b1, bmu, blv))
    nc, layout = _get_program(np.asarray(edge_index), cfg, has_bias)
    in_maps = make_in_maps(x, edge_index, W1, b1, Wmu, bmu, Wlv, blv,
                           layout, cfg, has_bias)
    res = run_bass_kernel_spmd(nc, in_maps, core_ids=list(range(cfg.NCORES)))
    return unshard(res.results, cfg)



# revision 69
# speedup vs baseline: 1.4326x; 1.4326x over previous
"""VGCN encoder (2-layer GCN, shared normalized adjacency) on 8 Trainium2 cores.

Strategy: node-partitioned graph parallelism. Nodes are padded to
NPAD = 8*6272 and core c owns nodes [6272c, 6272(c+1)), split into 98 buckets
of 64. All edges (plus one self-edge per node, which realizes both GCN's +1
degree and the self-loop term) are routed to the core owning their dst node,
bucketed by dst bucket, and aggregated on-device with one-hot matmuls over
128-edge tiles (bf16 operands, fp32 psum):

    agg[bucket] += S.T @ us[src]   (S[e, j] = dst_local[e] == j, built on DVE)

Messages are fetched with SWDGE dma_gather (4 queues round-robin) from a
bf16 DRAM table whose 256-byte rows ([HID bf16 payload | HID pad], the
minimum SWDGE element) are stored in a (core, partition, bucket)-major
permutation so bulk table writes are contiguous DMAs; the host permutes
gather indices to match. dis = 1/sqrt(deg) is precomputed on the host (pure
graph structure, like the edge bucketing itself).

Layer 1 input (x @ W1, x shipped in bf16) is computed REPLICATED on every
core for the whole graph - it is ~50 MFLOP, far cheaper than an AllGather
round - so the only collective is the AllGather of the layer-1 activations
(packed bf16, Shared-scratchpad output, expanded to padded rows locally).
Layer-2 aggregation runs feature-major (lhsT=msg) so both output projections
become 13 wide matmuls against [Wmu | Wlv] with one transpose per 128 nodes.

All host-prepared operands travel in ONE packed int16 input tensor (f32/bf16
sections bitcast on device) and both outputs leave in ONE tensor; together
with pipelined dispatch this keeps the per-execution dispatch overhead of the
axon tunnel (~0.8 ms flat per round trip) mostly off the measured
steady-state time.
"""

import sys

sys.path.insert(0, "/opt/trn_rl_repo")

import numpy as np

from concourse import bacc, mybir, tile
from concourse.bass_utils import run_bass_kernel_spmd
from concourse.masks import make_identity

F32 = mybir.dt.float32
BF16 = mybir.dt.bfloat16
I16 = mybir.dt.int16
I32 = mybir.dt.int32


class Cfg:
    def __init__(self, n=50000, e=800000, in_dim=128, hid=64, ncores=8,
                 shard_tiles=49, bw=64, half=32768, chunk_tiles=38, sbatch=16):
        self.N, self.E, self.IN, self.HID = n, e, in_dim, hid
        self.NCORES = ncores
        self.P = 128
        self.SHARD = shard_tiles * 128    # nodes per core
        self.NPAD = ncores * self.SHARD
        self.BW = bw                      # bucket width (psum partition dim)
        self.NBK = self.SHARD // bw       # buckets per core
        self.GBK = ncores * self.NBK      # global buckets
        self.HALF = half                  # gather-table split so int16 idx fit
        self.CH = chunk_tiles             # tiles (128 rows) per dma_gather
        self.SB = sbatch                  # tiles per batched one-hot build
        assert self.NPAD >= n and half <= 32768 and self.SHARD % bw == 0
        assert self.NBK <= 128 and self.NBK % 2 == 0


DEFAULT = Cfg()


def pack_offsets(cfg, nTA, nTB):
    """Column offsets (int16 units) of each section in the packed input."""
    off, out = 0, {}
    for name, width in (("xT", cfg.NPAD), ("w1", 2 * cfg.HID),
                        ("wml", 4 * cfg.HID), ("dis", 2 * cfg.NBK),
                        ("disP", cfg.NBK), ("dlA", nTA),
                        ("dlB", nTB), ("idxA", 8 * nTA),
                        ("idxB", 8 * nTB)):
        out[name] = off
        off += width
    out["W"] = off
    return out


def build_layout(edge_index, cfg=DEFAULT):
    """Static per-core edge streams plus the (identical-across-cores) tile
    structure. Table row of node n: c*SHARD + (r%BW)*NBK + r//BW, r=n%SHARD."""
    src = np.asarray(edge_index[0], np.int64)
    dst = np.asarray(edge_index[1], np.int64)
    NBK, BW = cfg.NBK, cfg.BW

    deg = np.bincount(dst, minlength=cfg.NPAD).astype(np.float64) + 1.0
    dis = (1.0 / np.sqrt(deg)).astype(np.float32)   # padding nodes: dis=1

    per_core = []
    cnts = np.zeros((cfg.NCORES, NBK * 2), np.int64)
    for c in range(cfg.NCORES):
        m = (dst >= c * cfg.SHARD) & (dst < (c + 1) * cfg.SHARD)
        s = src[m]
        d = dst[m]
        selfn = np.arange(c * cfg.SHARD, (c + 1) * cfg.SHARD, dtype=np.int64)
        s = np.concatenate([s, selfn])
        d = np.concatenate([d, selfn])
        cc, rr = s // cfg.SHARD, s % cfg.SHARD
        row = cc * cfg.SHARD + (rr % BW) * NBK + rr // BW
        dr = d - c * cfg.SHARD
        b = dr // BW
        dl = dr % BW
        h = (row >= cfg.HALF).astype(np.int64)
        key = b * 2 + h
        order = np.argsort(key, kind="stable")
        row, dl, key = row[order], dl[order], key[order]
        per_core.append((row, dl, key))
        cnts[c] = np.bincount(key, minlength=NBK * 2)

    ntile = np.ceil(cnts.max(axis=0) / 128.0).astype(np.int64)
    ntA, ntB = ntile[0::2], ntile[1::2]
    nTA, nTB = int(ntA.sum()), int(ntB.sum())

    tbA = np.repeat(np.arange(NBK), ntA)
    tbB = np.repeat(np.arange(NBK), ntB)
    offA = np.concatenate([[0], np.cumsum(ntA)]) * 128
    offB = np.concatenate([[0], np.cumsum(ntB)]) * 128

    cores = []
    for c in range(cfg.NCORES):
        row, dl, key = per_core[c]
        bounds = np.searchsorted(key, np.arange(NBK * 2 + 1))
        idxA = np.zeros(nTA * 128, np.int64)
        dlA = np.full(nTA * 128, BW, np.int64)
        idxB = np.zeros(nTB * 128, np.int64)
        dlB = np.full(nTB * 128, BW, np.int64)
        for b in range(NBK):
            lo, hi = bounds[2 * b], bounds[2 * b + 1]
            o = offA[b]
            idxA[o:o + hi - lo] = row[lo:hi]
            dlA[o:o + hi - lo] = dl[lo:hi]
            lo, hi = bounds[2 * b + 1], bounds[2 * b + 2]
            o = offB[b]
            idxB[o:o + hi - lo] = row[lo:hi] - cfg.HALF
            dlB[o:o + hi - lo] = dl[lo:hi]

        def wrap(stream):
            a = stream.reshape(-1, 16).T.astype(np.int16)
            return np.tile(a, (8, 1))   # replicated across the 8 q7 cores

        # dis for own nodes in (partition=dl, bucket) layout
        dis_own = np.ascontiguousarray(
            dis[c * cfg.SHARD:(c + 1) * cfg.SHARD].reshape(NBK, BW).T)

        disP = np.ascontiguousarray(
            dis_own.reshape(BW, NBK // 2, 2).transpose(2, 0, 1)
            .reshape(2 * BW, NBK // 2))

        cores.append(dict(
            idxA=wrap(idxA), idxB=wrap(idxB),
            dlA=np.ascontiguousarray(dlA.reshape(-1, 128).T.astype(np.float32)),
            dlB=np.ascontiguousarray(dlB.reshape(-1, 128).T.astype(np.float32)),
            dis=dis_own, disP=disP,
        ))

    return dict(ntA=tuple(int(x) for x in ntA), ntB=tuple(int(x) for x in ntB),
                tbA=tbA, tbB=tbB, nTA=nTA, nTB=nTB, cores=cores,
                dis_full=dis.astype(np.float32))


def build_program(layout, cfg=DEFAULT, has_bias=False, reps=1,
                  skip_cc=False, phases=3, gather_mode="gather", proj=True):
    """Emit the SPMD bass program (identical on all cores)."""
    nc = bacc.Bacc("TRN2", target_bir_lowering=False, debug=False,
                   num_devices=cfg.NCORES, num_swdge_queues=4)
    P, BW, NBK, HID = cfg.P, cfg.BW, cfg.NBK, cfg.HID
    nTA, nTB = layout["nTA"], layout["nTB"]
    tb = {0: layout["tbA"], 1: layout["tbB"]}
    nT = {0: nTA, 1: nTB}
    HALVES = [H for H in (0, 1) if nT[H] > 0]
    use_cc = cfg.NCORES > 1 and not skip_cc

    # ---------------- I/O (single packed input, single output) ----------
    OFF = pack_offsets(cfg, nTA, nTB)
    pk_in = nc.dram_tensor("pk", [P, OFF["W"]], I16, kind="ExternalInput")

    def fsec(name, n, rows=P):
        o = OFF[name]
        return pk_in.ap()[:rows, o:o + 2 * n].bitcast(F32)

    def bsec(name, n, rows=P):
        o = OFF[name]
        return pk_in.ap()[:rows, o:o + n].bitcast(BF16)

    def isec(name, n):
        o = OFF[name]
        return pk_in.ap()[:, o:o + n]

    xT_b = pk_in.ap()[:, OFF["xT"]:OFF["xT"] + cfg.NPAD].bitcast(BF16)
    if has_bias:
        b1_in = nc.dram_tensor("b1", [1, HID], F32, kind="ExternalInput")
        bmu_in = nc.dram_tensor("bmu", [1, HID], F32, kind="ExternalInput")
        blv_in = nc.dram_tensor("blv", [1, HID], F32, kind="ExternalInput")
    z_out = nc.dram_tensor("z", [P, NBK // 2, 2 * HID], BF16,
                           kind="ExternalOutput")

    with tile.TileContext(nc) as tc:
        import contextlib
        stack = contextlib.ExitStack()
        with stack:
            dram = stack.enter_context(tc.tile_pool(name="dram", bufs=1, space="DRAM"))
            cpool = stack.enter_context(tc.tile_pool(name="const", bufs=1))

            us_tab = dram.tile([cfg.NPAD, 2 * HID], BF16)
            hs2_bnc = dram.tile([cfg.SHARD, HID], BF16)
            hs2_pk = dram.tile([cfg.NPAD, HID], BF16, addr_space="Shared")
            hs2_tab = dram.tile([cfg.NPAD, 2 * HID], BF16)

            w1_f = cpool.tile([cfg.IN, HID], F32)
            nc.sync.dma_start(out=w1_f[:], in_=fsec("w1", HID))
            w1_sb = cpool.tile([cfg.IN, HID], BF16)
            nc.vector.tensor_copy(out=w1_sb[:], in_=w1_f[:])
            wml_f = cpool.tile([HID, 2 * HID], F32)
            nc.sync.dma_start(out=wml_f[:], in_=fsec("wml", 2 * HID, HID))
            wml_sb = cpool.tile([HID, 2 * HID], BF16)
            nc.vector.tensor_copy(out=wml_sb[:], in_=wml_f[:])
            dis_own = cpool.tile([BW, NBK], F32)
            nc.sync.dma_start(out=dis_own[:], in_=fsec("dis", NBK, BW))
            disP = cpool.tile([P, NBK // 2], F32)
            nc.sync.dma_start(out=disP[:], in_=fsec("disP", NBK // 2))
            dis2 = cpool.tile([BW, NBK], F32)
            nc.vector.tensor_tensor(out=dis2[:], in0=dis_own[:],
                                    in1=dis_own[:], op=mybir.AluOpType.mult)

            iota_i = cpool.tile([P, BW], I32)
            nc.gpsimd.iota(iota_i[:], pattern=[[1, BW]], base=0,
                           channel_multiplier=0)
            iota_b = cpool.tile([P, BW], BF16)
            nc.vector.tensor_copy(out=iota_b[:], in_=iota_i[:])

            ident = cpool.tile([P, P], F32)
            make_identity(nc, ident[:])
            ident_bf = cpool.tile([P, P], BF16)
            nc.vector.tensor_copy(out=ident_bf[:], in_=ident[:])

            idx_sb, dl_sb = {}, {}
            for H in HALVES:
                idx_sb[H] = cpool.tile([P, nT[H] * 8], I16, tag=f"idx{H}",
                                       name=f"idx{H}")
                nc.sync.dma_start(out=idx_sb[H][:],
                                  in_=isec("idxA" if H == 0 else "idxB",
                                           nT[H] * 8))
                dl_sb[H] = cpool.tile([P, nT[H]], BF16, tag=f"dl{H}",
                                      name=f"dls{H}")
                nc.sync.dma_start(out=dl_sb[H][:],
                                  in_=bsec("dlA" if H == 0 else "dlB",
                                           nT[H]))

            if has_bias:
                brow = cpool.tile([1, 3 * HID], F32)
                nc.sync.dma_start(out=brow[:, 0:HID], in_=b1_in.ap()[:])
                nc.sync.dma_start(out=brow[:, HID:2 * HID], in_=bmu_in.ap()[:])
                nc.sync.dma_start(out=brow[:, 2 * HID:], in_=blv_in.ap()[:])
                bias_bc = cpool.tile([P, 3 * HID], F32)
                nc.gpsimd.partition_broadcast(bias_bc[:], brow[:])

            def build_S(spool, H, tag):
                tiles = []
                for t0 in range(0, nT[H], cfg.SB):
                    tn = min(cfg.SB, nT[H] - t0)
                    st = spool.tile([P, cfg.SB, BW], BF16, tag=tag,
                                    name=f"S{tag}")
                    nc.vector.tensor_tensor(
                        out=st[:, :tn, :],
                        in0=dl_sb[H][:, t0:t0 + tn].to_broadcast([P, tn, BW]),
                        in1=iota_b[:, None, :].to_broadcast([P, tn, BW]),
                        op=mybir.AluOpType.is_equal,
                    )
                    tiles.append(st)

                def one(t):
                    return tiles[t // cfg.SB][:, t % cfg.SB, :]

                return one

            qctr = [0]

            def gather_all(mpool, table, tagp):
                """Gather both halves' chunks, interleaved so the earliest
                tiles of BOTH halves land first (bucket 0 needs both)."""
                tiles = {H: [] for H in HALVES}
                starts = {H: list(range(0, nT[H], cfg.CH)) for H in HALVES}
                order = []
                for i in range(max(len(starts[H]) for H in HALVES)):
                    for H in HALVES:
                        if i < len(starts[H]):
                            order.append((H, starts[H][i]))
                for H, t0 in order:
                    tn = min(cfg.CH, nT[H] - t0)
                    mt = mpool.tile([P, cfg.CH, 2 * HID], BF16,
                                    tag=f"{tagp}{H}", name=f"M{tagp}{H}")
                    if gather_mode == "copy":
                        nc.sync.dma_start(
                            out=mt[:, :tn, :],
                            in_=table[:tn * 128, :]
                            .rearrange("(t p) f -> p t f", p=128))
                    else:
                        nc.gpsimd.dma_gather(
                            out_ap=mt[:, :tn, :],
                            in_ap=(table[:min(cfg.HALF, cfg.NPAD), :]
                                   if H == 0 else table[cfg.HALF:, :]),
                            idxs_ap=idx_sb[H][:, t0 * 8:(t0 + tn) * 8],
                            num_idxs=tn * 128, num_idxs_reg=tn * 128,
                            elem_size=2 * HID,
                            single_packet=(tn * 128 <= 512),
                            queue_num=qctr[0] % 4,
                        )
                        qctr[0] += 1
                    tiles[H].append(mt)

                def make(H):
                    tl = tiles[H]
                    return lambda t: tl[t // cfg.CH][:, t % cfg.CH, 0:HID]

                return {H: make(H) for H in HALVES}

            entries = [[] for _ in range(NBK)]
            for H in HALVES:
                for t, b in enumerate(tb[H]):
                    entries[int(b)].append((H, t))

            for _rep in range(reps):
                # ========= PHASE A: u = x @ W1, scaled by dis -> us table ====
                it_stack = contextlib.ExitStack()
                with it_stack:
                    xa = it_stack.enter_context(tc.tile_pool(name="xa", bufs=2))
                    usb = it_stack.enter_context(tc.tile_pool(name="usb", bufs=2))
                    spool = it_stack.enter_context(tc.tile_pool(name="spool", bufs=2))
                    mpool = it_stack.enter_context(tc.tile_pool(name="mpool", bufs=6))
                    hb = it_stack.enter_context(tc.tile_pool(name="hb", bufs=2))

                    XC = 8   # buckets per psum bank / ACT copy batch
                    XL = 49  # buckets per xT DMA (2 loads per shard)
                    if phases < 1:
                        zfill0 = usb.tile([BW, NBK, HID], BF16, tag="usbb",
                                          name="zfill0")
                        nc.vector.memset(zfill0[:], 0)
                    c2_order = [0, 1, 2, 3, 4, 5, 6, 7][:cfg.NCORES]
                    a_stack = contextlib.ExitStack()
                    pu = a_stack.enter_context(
                        tc.tile_pool(name="pu", bufs=4, space="PSUM"))
                    for c2 in (c2_order if phases >= 1 else []):
                        us_blk = usb.tile([BW, NBK, HID], BF16, tag="usbb",
                                          name="us_blk")
                        xtbs = {}
                        for L0 in range(0, NBK, XL):
                            ln = min(XL, NBK - L0)
                            xtb = xa.tile([P, XL, BW], BF16, tag="xtb",
                                          name="xtb")
                            nc.sync.dma_start(
                                out=xtb[:, :ln, :],
                                in_=xT_b[:, c2 * cfg.SHARD + L0 * BW:
                                         c2 * cfg.SHARD + (L0 + ln) * BW]
                                .rearrange("p (t q) -> p t q", t=ln))
                            xtbs[L0] = xtb
                        for B0 in range(0, NBK, XC):
                            bn = min(XC, NBK - B0)
                            ups = pu.tile([BW, XC, HID], F32, space="PSUM",
                                          tag="u", name="ups")
                            for j in range(bn):
                                L0 = ((B0 + j) // XL) * XL
                                nc.tensor.matmul(out=ups[:, j, :],
                                                 lhsT=xtbs[L0][:, B0 + j - L0, :],
                                                 rhs=w1_sb[:],
                                                 start=True, stop=True)
                            # dis[src] is folded into x on the host, so this
                            # is a pure psum->sbuf bf16 cast (ACT engine).
                            nc.scalar.copy(out=us_blk[:, B0:B0 + bn, :],
                                           in_=ups[:, :bn, :])
                        nc.sync.dma_start(
                            out=us_tab[c2 * cfg.SHARD:(c2 + 1) * cfg.SHARD,
                                       0:HID]
                            .rearrange("(j b) f -> j b f", j=BW),
                            in_=us_blk[:])
                    a_stack.close()
                    pagg = it_stack.enter_context(
                        tc.tile_pool(name="pagg", bufs=3, space="PSUM"))
                    pproj = it_stack.enter_context(
                        tc.tile_pool(name="pproj", bufs=2, space="PSUM"))
                    pz = it_stack.enter_context(
                        tc.tile_pool(name="pz", bufs=1, space="PSUM"))

                    # ================= PHASE B: layer-1 aggregation =============
                    if phases >= 2:
                        msg = gather_all(mpool, us_tab[:], "m")
                        S1 = {H: build_S(spool, H, f"s{H}") for H in HALVES}
                        hs2_sb = usb.tile([BW, NBK, HID], BF16, tag="usbb",
                                          name="hs2_sb")
                        for b0 in range(0, NBK, 2):
                            ps = pagg.tile([BW, 2, HID], F32, space="PSUM",
                                           tag="agg", name="ps1")
                            for k in (0, 1):
                                ent = entries[b0 + k]
                                for i, (H, t) in enumerate(ent):
                                    nc.tensor.matmul(
                                        out=ps[:, k, :], lhsT=S1[H](t),
                                        rhs=msg[H](t), start=(i == 0),
                                        stop=(i == len(ent) - 1))
                            if has_bias:
                                t1 = hb.tile([BW, 2, HID], F32, tag="h",
                                             name="t1")
                                # h1 = relu(ps*dis + b); hs2 = h1*dis
                                dpair = dis_own[:, b0:b0 + 2, None] \
                                    .to_broadcast([BW, 2, HID])
                                nc.vector.tensor_tensor(
                                    out=t1[:], in0=ps[:], in1=dpair,
                                    op=mybir.AluOpType.mult)
                                nc.vector.tensor_tensor(
                                    out=t1[:], in0=t1[:],
                                    in1=bias_bc[:BW, None, 0:HID]
                                    .to_broadcast([BW, 2, HID]),
                                    op=mybir.AluOpType.add)
                                nc.vector.tensor_relu(out=t1[:], in_=t1[:])
                                nc.vector.tensor_tensor(
                                    out=hs2_sb[:, b0:b0 + 2, :], in0=t1[:],
                                    in1=dpair, op=mybir.AluOpType.mult)
                            else:
                                # dis>0: relu(ps)*dis^2 == relu(ps*dis^2);
                                # one ACT op per bucket keeps DVE free for
                                # the S-matrix builds.
                                for k in (0, 1):
                                    nc.scalar.activation(
                                        out=hs2_sb[:, b0 + k, :],
                                        in_=ps[:, k, :],
                                        func=mybir.ActivationFunctionType.Relu,
                                        scale=dis2[:, b0 + k, None])
                        if use_cc:
                            nc.sync.dma_start(
                                out=hs2_bnc[:].rearrange("(j b) f -> j b f", j=BW),
                                in_=hs2_sb[:])
                            nc.gpsimd.collective_compute(
                                "AllGather", mybir.AluOpType.bypass,
                                replica_groups=[list(range(cfg.NCORES))],
                                ins=[hs2_bnc.opt()], outs=[hs2_pk.opt()],
                            )
                            nc.sync.dma_start(out=hs2_tab[:, 0:HID],
                                              in_=hs2_pk[:])
                        else:
                            nc.sync.dma_start(
                                out=hs2_pk[:cfg.SHARD, :]
                                .rearrange("(j b) f -> j b f", j=BW),
                                in_=hs2_sb[:])
                            nc.sync.dma_start(out=hs2_tab[:, 0:HID],
                                              in_=hs2_pk[:])

                    # ============== PHASE C: layer-2 + projections ==============
                    if phases >= 3:
                        msg = gather_all(mpool, hs2_tab[:], "m")
                        S2 = {H: build_S(spool, H, f"s{H}") for H in HALVES}
                        # feature-major aggregation: aggT[f, node]
                        a2T_sb = usb.tile([HID, NBK * BW], BF16, tag="usb",
                                          name="a2T_sb")
                        for b0 in range(0, NBK, 2):
                            ps = pagg.tile([HID, 2, BW], F32, space="PSUM",
                                           tag="agg", name="ps2")
                            for k in (0, 1):
                                ent = entries[b0 + k]
                                for i, (H, t) in enumerate(ent):
                                    nc.tensor.matmul(
                                        out=ps[:, k, :], lhsT=msg[H](t),
                                        rhs=S2[H](t), start=(i == 0),
                                        stop=(i == len(ent) - 1))
                            nc.scalar.copy(
                                out=a2T_sb[:, b0 * BW:(b0 + 2) * BW],
                                in_=ps[:])
                        if not proj:
                            nc.sync.dma_start(
                                out=z_out.ap()[0:HID, :, :],
                                in_=a2T_sb[:].rearrange(
                                    "p (t q) -> p t q", q=2 * HID))
                        # zcatT = [Wmu | Wlv].T @ aggT  -> [2H, nodes]
                        zT_sb = usb.tile([2 * HID, NBK * BW], BF16, tag="usb",
                                         name="zT_sb")
                        CHK = 512
                        for n0 in (range(0, NBK * BW, CHK) if proj else []):
                            cn = min(CHK, NBK * BW - n0)
                            zT_ps = pproj.tile([2 * HID, CHK], F32,
                                               space="PSUM", tag="zT",
                                               name="zT_ps")
                            nc.tensor.matmul(out=zT_ps[:, :cn],
                                             lhsT=wml_sb[:],
                                             rhs=a2T_sb[:, n0:n0 + cn],
                                             start=True, stop=True)
                            nc.scalar.copy(out=zT_sb[:, n0:n0 + cn],
                                           in_=zT_ps[:, :cn])
                        # transpose back per 128-node pair, scale by dis
                        zcat_sb = usb.tile([P, NBK // 2, 2 * HID], BF16,
                                           tag="usb", name="zcat_sb")
                        for t in (range(NBK // 2) if proj else []):
                            z_ps = pz.tile([P, P], BF16, space="PSUM",
                                           tag="z", name="z_ps")
                            nc.tensor.transpose(
                                out=z_ps[:],
                                in_=zT_sb[:, t * P:(t + 1) * P],
                                identity=ident_bf[:])
                            nc.scalar.mul(out=zcat_sb[:, t, :], in_=z_ps[:],
                                          mul=disP[:, t, None])
                        if has_bias and proj:
                            nc.vector.tensor_tensor(
                                out=zcat_sb[:], in0=zcat_sb[:],
                                in1=bias_bc[:, None, HID:3 * HID]
                                .to_broadcast([P, NBK // 2, 2 * HID]),
                                op=mybir.AluOpType.add)
                        if proj:
                            nc.sync.dma_start(out=z_out.ap()[:],
                                              in_=zcat_sb[:])
                    if phases < 3:
                        zfill = usb.tile([P, NBK // 2, 2 * HID], BF16,
                                         tag="usb", name="zfill")
                        nc.vector.memset(zfill[:], 0)
                        nc.sync.dma_start(out=z_out.ap()[:], in_=zfill[:])

    nc.compile()
    return nc


_CACHE = {}


def _get_program(edge_index, cfg, has_bias):
    layout = build_layout(edge_index, cfg)
    key = (layout["ntA"], layout["ntB"], has_bias)
    if key not in _CACHE:
        _CACHE[key] = build_program(layout, cfg, has_bias)
    return _CACHE[key], layout


def make_in_maps(x, edge_index, W1, b1, Wmu, bmu, Wlv, blv, layout,
                 cfg=DEFAULT, has_bias=False):
    x = np.asarray(x, np.float32)
    xpad = np.zeros((cfg.NPAD, cfg.IN), np.float32)
    xpad[:x.shape[0]] = x
    # fold dis[src] into x so phase A's matmul directly yields us = dis*(x@W1)
    xpad *= layout["dis_full"][:, None]
    xT = np.ascontiguousarray(xpad.T)
    wml = np.concatenate([np.asarray(Wmu, np.float32),
                          np.asarray(Wlv, np.float32)], axis=1)
    w1 = np.asarray(W1, np.float32)
    nTA, nTB = layout["nTA"], layout["nTB"]
    OFF = pack_offsets(cfg, nTA, nTB)

    def put_f32(pk, name, arr):
        arr = np.asarray(arr, np.float32)
        o = OFF[name]
        pk[:arr.shape[0], o:o + 2 * arr.shape[1]] = arr.view(np.int16)

    def put_bf16(pk, name, arr):
        import ml_dtypes
        arr = np.asarray(arr, np.float32).astype(ml_dtypes.bfloat16)
        o = OFF[name]
        pk[:arr.shape[0], o:o + arr.shape[1]] = arr.view(np.int16)

    maps = []
    for c in range(cfg.NCORES):
        pk = np.zeros((cfg.P, OFF["W"]), np.int16)
        cd = layout["cores"][c]
        put_bf16(pk, "xT", xT)
        put_f32(pk, "w1", w1)
        put_f32(pk, "wml", wml)
        put_f32(pk, "dis", cd["dis"])
        put_f32(pk, "disP", cd["disP"])
        put_bf16(pk, "dlA", cd["dlA"])
        put_bf16(pk, "dlB", cd["dlB"])
        pk[:, OFF["idxA"]:OFF["idxA"] + 8 * nTA] = cd["idxA"]
        pk[:, OFF["idxB"]:OFF["idxB"] + 8 * nTB] = cd["idxB"]
        m = dict(pk=pk)
        if has_bias:
            m.update(b1=np.asarray(b1, np.float32).reshape(1, -1),
                     bmu=np.asarray(bmu, np.float32).reshape(1, -1),
                     blv=np.asarray(blv, np.float32).reshape(1, -1))
        maps.append(m)
    return maps


def unshard(results, cfg=DEFAULT):
    H = cfg.HID
    zmu_blocks, zlv_blocks = [], []
    for c in range(cfg.NCORES):
        z = np.asarray(results[c]["z"]).astype(np.float32)
        z4 = z.reshape(2, cfg.BW, cfg.NBK // 2, 2 * H)
        zjb = np.transpose(z4, (2, 0, 1, 3)).reshape(cfg.NBK, cfg.BW, 2 * H)
        # zjb[b, j, :]: node c*SHARD + b*BW + j
        zmu_blocks.append(zjb[:, :, 0:H].reshape(cfg.SHARD, H))
        zlv_blocks.append(zjb[:, :, H:2 * H].reshape(cfg.SHARD, H))
    return (np.concatenate(zmu_blocks, axis=0)[:cfg.N],
            np.concatenate(zlv_blocks, axis=0)[:cfg.N])


def kernel(x, edge_index, W1, b1, Wmu, bmu, Wlv, blv):
    cfg = DEFAULT
    has_bias = any(np.any(np.asarray(b)) for b in (b1, bmu, blv))
    nc, layout = _get_program(np.asarray(edge_index), cfg, has_bias)
    in_maps = make_in_maps(x, edge_index, W1, b1, Wmu, bmu, Wlv, blv,
                           layout, cfg, has_bias)
    res = run_bass_kernel_spmd(nc, in_maps, core_ids=list(range(cfg.NCORES)))
    return unshard(res.results, cfg)

